# revision 8
# baseline (speedup 1.0000x reference)
"""Chamfer loss kernel for 8 Trainium2 NeuronCores (Bass/Tile).

Problem: x [4,4096,3], y [4,4096,3] fp32 ->
  scalar = mean_m min_n ||x_n - y_m|| + mean_n min_m ||x_n - y_m||  (per batch, averaged)

Strategy
--------
s[m,n] = -||x_n - y_m||^2 = 2<y_m,x_n> - ||y_m||^2 - ||x_n||^2 = <yhat_m, xhat_n>
with yhat = [2y, -||y||^2, -1], xhat = [x, 1, ||x||^2]  (Da = 5).
Each side is split 3-way into bf16 (h1+h2+h3 ~ fp32 accurate); all 9 block
products are stacked along the matmul contraction axis -> K = 45 <= 128, so
the PE computes fp32-grade s tiles at bf16 speed (cost ~ N columns).

Both chamfer directions are max-reductions of s (sqrt is monotonic, applied on
host to the reduced values only):
  dist1[m] = max_n s[m,n]   (free-axis  -> one fused tensor_tensor_reduce per m-tile)
  dist2[n] = max_m s[m,n]   (partition/tile axis -> fp16 tensor_max chains, then
                             PE-transpose + reduce_max tail)

Sharding: core c -> batch b = c//2, m-half h = c%2 (2048 m x 4096 n per core).
dist1 is exact per core; dist2 partials are max-combined on host.
"""

import sys

if "/opt/trn_rl_repo" not in sys.path:
    sys.path.insert(0, "/opt/trn_rl_repo")

from contextlib import ExitStack

import numpy as np
import ml_dtypes

import concourse.bass as bass
import concourse.tile as tile
from concourse import bacc, mybir
from concourse.bass_utils import run_bass_kernel_spmd
from concourse.masks import make_identity

B, N, M, D = 4, 4096, 4096, 3
DA = 5          # augmented vector length
KS = 3 * 3 * DA  # 45: 3x3 split products stacked on contraction axis
MT = 16         # m-tiles of 128 per core (2048 m's)
NT = 8          # n-tiles of 512
NG = 2          # groups of 4 n-tiles (2048 n's each)
GW = 4 * 512    # group width
P = 128

FP32 = mybir.dt.float32
FP16 = mybir.dt.float16
BF16 = mybir.dt.bfloat16
NEG_INF = -3.0e38


def build_program(repeat: int = 1, use_ttr: bool = False, mt: int = MT):
    """Build the SPMD bass program. Returns compiled Bacc object."""
    nc = bacc.Bacc("TRN2", target_bir_lowering=False, debug=False, num_devices=8)

    ys_d = nc.dram_tensor("ys", [KS, MT * P], BF16, kind="ExternalInput").ap()
    xs_d = nc.dram_tensor("xs", [KS, N], BF16, kind="ExternalInput").ap()
    out_d = nc.dram_tensor("out", [P, MT + 32], FP32, kind="ExternalOutput").ap()

    with tile.TileContext(nc) as tc, ExitStack() as ctx:
        consts = ctx.enter_context(tc.tile_pool(name="consts", bufs=1))
        y_sb = consts.tile([KS, MT * P], BF16, tag="y_sb")
        x_sb = consts.tile([KS, N], BF16, tag="x_sb")
        nc.sync.dma_start(y_sb[:], ys_d[:])
        nc.sync.dma_start(x_sb[:], xs_d[:])

        d1 = consts.tile([P, MT], FP32, tag="d1")       # dist1: col t
        d2 = consts.tile([P, 32], FP32, tag="d2")       # dist2: col G*16+c

        ident = consts.tile([P, P], FP32, tag="ident")
        make_identity(nc, ident[:])

        acc2 = [
            consts.tile([P, GW], FP16, tag=f"acc2_{g}", name=f"acc2_{g}")
            for g in range(NG)
        ]

        for _rep in range(repeat):
            with (
                tc.tile_pool(name="psum", bufs=2, space="PSUM") as psum_pool,
                tc.tile_pool(name="cpool", bufs=4) as cpool,
                tc.tile_pool(name="junk", bufs=2) as junkpool,
            ):
                for t in range(mt):
                    lhs = y_sb[:, t * P:(t + 1) * P]
                    ctiles = []
                    for g in range(NG):
                        ps = psum_pool.tile([P, GW], FP32)
                        for k in range(4):
                            nc.tensor.matmul(
                                ps[:, k * 512:(k + 1) * 512],
                                lhsT=lhs,
                                rhs=x_sb[:, (g * 4 + k) * 512:(g * 4 + k + 1) * 512],
                                start=True,
                                stop=True,
                            )
                        c = cpool.tile([P, GW], FP16)
                        nc.scalar.copy(c[:], ps[:])
                        ctiles.append(c)
                        # dist2 chain: acc2_g = max(acc2_g, c)
                        if _rep == 0 and t == 0:
                            nc.vector.tensor_copy(acc2[g][:], c[:])
                        else:
                            nc.vector.tensor_max(acc2[g][:], acc2[g][:], c[:])
                    # dist1 funnel: fp16 fold chain (2x_1p) then one reduce.
                    jk = junkpool.tile([P, GW], FP16)
                    if use_ttr:
                        nc.vector.tensor_tensor_reduce(
                            out=jk[:],
                            in0=ctiles[0][:],
                            in1=ctiles[1][:],
                            scale=1.0,
                            scalar=NEG_INF,
                            op0=mybir.AluOpType.max,
                            op1=mybir.AluOpType.max,
                            accum_out=d1[:, t:t + 1],
                        )
                    else:
                        nc.vector.tensor_max(jk[:], ctiles[0][:], ctiles[1][:])
                        nc.vector.tensor_max(
                            jk[:, 0:1024], jk[:, 0:1024], jk[:, 1024:2048]
                        )
                        nc.vector.tensor_max(
                            jk[:, 0:512], jk[:, 0:512], jk[:, 512:1024]
                        )
                        nc.vector.tensor_reduce(
                            d1[:, t:t + 1],
                            jk[:, 0:512],
                            axis=mybir.AxisListType.X,
                            op=mybir.AluOpType.max,
                        )

        # ---- dist2 tail: partition-axis max of acc2 groups ----
        # acc2[g] [128, 2048] fp16 -> fp32 -> PE-transpose 128x128 chunks into
        # one-bank psum tiles (4 chunks per tile), reduce_max over [P,4,P].
        with (
            tc.tile_pool(name="acc32", bufs=2) as acc32pool,
            tc.tile_pool(name="tpsum", bufs=3, space="PSUM") as tpsum,
        ):
            for g in range(NG):
                a32 = acc32pool.tile([P, GW], FP32)
                nc.scalar.copy(a32[:], acc2[g][:])
                for half in range(4):
                    pt = tpsum.tile([P, 512], FP32)
                    for k in range(4):
                        cc = half * 4 + k
                        nc.tensor.transpose(
                            pt[:, k * P:(k + 1) * P],
                            a32[:, cc * P:(cc + 1) * P],
                            ident[:],
                        )
                    nc.vector.tensor_reduce(
                        d2[:, g * 16 + half * 4: g * 16 + half * 4 + 4],
                        pt[:].rearrange("p (c q) -> p c q", c=4),
                        axis=mybir.AxisListType.X,
                        op=mybir.AluOpType.max,
                    )

        nc.sync.dma_start(out_d[:, 0:MT], d1[:])
        nc.sync.dma_start(out_d[:, MT:MT + 32], d2[:])

    nc.compile()
    return nc


def _np3split(v: np.ndarray):
    """3-way bf16 split of float64/float32 array v: returns (h1,h2,h3) bf16."""
    v = v.astype(np.float64)
    h1 = v.astype(ml_dtypes.bfloat16)
    r1 = v - h1.astype(np.float64)
    h2 = r1.astype(ml_dtypes.bfloat16)
    r2 = r1 - h2.astype(np.float64)
    h3 = r2.astype(ml_dtypes.bfloat16)
    return h1, h2, h3


def make_inputs(x: np.ndarray, y: np.ndarray):
    """Host prep: augmented, 3-way-split, K-stacked operands per core."""
    x = np.asarray(x, dtype=np.float32)
    y = np.asarray(y, dtype=np.float32)
    x64 = x.astype(np.float64)
    y64 = y.astype(np.float64)
    x2 = (x64 * x64).sum(-1)  # [B,N]
    y2 = (y64 * y64).sum(-1)  # [B,M]

    # xhat [B,DA,N], yhat [B,DA,M]
    xhat = np.empty((B, DA, N), np.float64)
    xhat[:, 0:3, :] = x64.transpose(0, 2, 1)
    xhat[:, 3, :] = 1.0
    xhat[:, 4, :] = x2
    yhat = np.empty((B, DA, M), np.float64)
    yhat[:, 0:3, :] = 2.0 * y64.transpose(0, 2, 1)
    yhat[:, 3, :] = -y2
    yhat[:, 4, :] = -1.0

    xh = _np3split(xhat)  # each [B,DA,N] bf16
    yh = _np3split(yhat)

    # K-stack: all 9 (i,j) products
    xs = np.empty((B, KS, N), ml_dtypes.bfloat16)
    ys = np.empty((B, KS, M), ml_dtypes.bfloat16)
    blk = 0
    for i in range(3):
        for j in range(3):
            ys[:, blk * DA:(blk + 1) * DA, :] = yh[i]
            xs[:, blk * DA:(blk + 1) * DA, :] = xh[j]
            blk += 1

    in_maps = []
    for c in range(8):
        b, h = c // 2, c % 2
        in_maps.append({
            "ys": np.ascontiguousarray(ys[b, :, h * 2048:(h + 1) * 2048]),
            "xs": np.ascontiguousarray(xs[b]),
        })
    return in_maps


def combine(results):
    """Host combine: results[c]["out"] [128, 48] -> scalar fp32."""
    smax1 = np.empty((B, M), np.float64)  # max_n s  (dist1 dir)
    smax2 = np.full((B, N), -np.inf, np.float64)  # max_m s (dist2 dir)
    for c in range(8):
        b, h = c // 2, c % 2
        o = np.asarray(results[c]["out"], np.float64)  # [128, 48]
        # D1: col t -> m = h*2048 + t*128 + p
        d1 = o[:, :MT]  # [128,16]
        smax1[b, h * 2048:(h + 1) * 2048] = d1.T.reshape(-1)
        # D2: col 16 + G*16+cc -> n = G*2048 + cc*128 + p
        d2 = o[:, MT:MT + 32]  # [128,32]
        smax2[b] = np.maximum(smax2[b], d2.T.reshape(-1))
    d2min_m = np.maximum(-smax1, 0.0)
    d2min_n = np.maximum(-smax2, 0.0)
    loss = np.sqrt(d2min_m).mean() + np.sqrt(d2min_n).mean()
    return np.float32(loss)


_CACHE = {}


def kernel(x, y):
    if "nc" not in _CACHE:
        _CACHE["nc"] = build_program(repeat=1)
    nc = _CACHE["nc"]
    in_maps = make_inputs(x, y)
    res = run_bass_kernel_spmd(nc, in_maps, list(range(8)))
    return combine(res.results)


# revision 18
# speedup vs baseline: 359.2850x; 359.2850x over previous
"""Chamfer loss kernel for 8 Trainium2 NeuronCores (Bass/Tile).

Problem: x [4,4096,3], y [4,4096,3] fp32 ->
  scalar = mean_m min_n ||x_n - y_m|| + mean_n min_m ||x_n - y_m||  (per batch, averaged)

Strategy
--------
s[m,n] = -||x_n - y_m||^2 = 2<y_m,x_n> - ||y_m||^2 - ||x_n||^2 = <yhat_m, xhat_n>
with yhat = [2y, -||y||^2, -1], xhat = [x, 1, ||x||^2]  (Da = 5).
Each side is split 3-way into bf16 (h1+h2+h3 ~ fp32 accurate); all 9 block
products are stacked along the matmul contraction axis -> K = 45 <= 128, so
the PE computes fp32-grade s tiles at bf16 speed (cost ~ N columns).

Both chamfer directions are max-reductions of s (sqrt is monotonic, applied on
host to the reduced values only):
  dist1[m] = max_n s[m,n]   (free-axis  -> one fused tensor_tensor_reduce per m-tile)
  dist2[n] = max_m s[m,n]   (partition/tile axis -> fp16 tensor_max chains, then
                             PE-transpose + reduce_max tail)

Sharding: core c -> batch b = c//2, m-half h = c%2 (2048 m x 4096 n per core).
dist1 is exact per core; dist2 partials are max-combined on host.
"""

import sys

if "/opt/trn_rl_repo" not in sys.path:
    sys.path.insert(0, "/opt/trn_rl_repo")

from contextlib import ExitStack

import numpy as np
import ml_dtypes

import concourse.bass as bass
import concourse.tile as tile
from concourse import bacc, mybir
from concourse.bass_utils import run_bass_kernel_spmd
from concourse.masks import make_identity

B, N, M, D = 4, 4096, 4096, 3
DA = 5          # augmented vector length
KS = 3 * 3 * DA  # 45: 3x3 split products stacked on contraction axis
MT = 16         # m-tiles of 128 per core (2048 m's)
NT = 8          # n-tiles of 512
NG = 2          # groups of 4 n-tiles (2048 n's each)
GW = 4 * 512    # group width
P = 128

FP32 = mybir.dt.float32
FP16 = mybir.dt.float16
BF16 = mybir.dt.bfloat16
NEG_INF = -3.0e38


def build_program(repeat: int = 1, use_ttr: bool = False, mt: int = MT, probe=()):
    """Build the SPMD bass program. Returns compiled Bacc object."""
    nc = bacc.Bacc("TRN2", target_bir_lowering=False, debug=False, num_devices=8)

    ys_d = nc.dram_tensor("ys", [KS, MT * P], BF16, kind="ExternalInput").ap()
    xs_d = nc.dram_tensor("xs", [KS, N], BF16, kind="ExternalInput").ap()
    out_d = nc.dram_tensor("out", [P, MT], FP32, kind="ExternalOutput").ap()
    acc_d = nc.dram_tensor("acc", [P, N], FP16, kind="ExternalOutput").ap()

    with tile.TileContext(nc) as tc, ExitStack() as ctx:
        consts = ctx.enter_context(tc.tile_pool(name="consts", bufs=1))
        y_sb = consts.tile([KS, MT * P], BF16, tag="y_sb")
        x_sb = consts.tile([KS, N], BF16, tag="x_sb")
        nc.sync.dma_start(y_sb[:], ys_d[:])
        nc.sync.dma_start(x_sb[:], xs_d[:])

        d1 = consts.tile([P, MT], FP32, tag="d1")       # dist1: col t
        # folded dist1 rows: col block t holds 512-wide folded maxima
        w1 = consts.tile([P, MT * 512], FP16, tag="w1")
        # dist2 accumulator: col f = n; partition-axis max finished on host
        acc2 = consts.tile([P, N], FP16, tag="acc2")

        for _rep in range(repeat):
            with (
                tc.tile_pool(name="psum", bufs=2, space="PSUM") as psum_pool,
                tc.tile_pool(name="cpool", bufs=4) as cpool,
                tc.tile_pool(name="junk", bufs=2) as junkpool,
            ):
                for t in range(mt):
                    lhs = y_sb[:, t * P:(t + 1) * P]
                    c = cpool.tile([P, N], FP16)
                    for g in range(NG):
                        ps = psum_pool.tile([P, GW], FP32)
                        for k in range(4):
                            nc.tensor.matmul(
                                ps[:, k * 512:(k + 1) * 512],
                                lhsT=lhs,
                                rhs=x_sb[:, (g * 4 + k) * 512:(g * 4 + k + 1) * 512],
                                start=True,
                                stop=True,
                            )
                        if "nocopy" not in probe:
                            nc.scalar.copy(c[:, g * GW:(g + 1) * GW], ps[:])
                    # dist2 chain: one wide fp16 op over all 4096 n's
                    if "nochain" in probe:
                        pass
                    elif _rep == 0 and t == 0:
                        nc.vector.tensor_copy(acc2[:], c[:])
                    else:
                        nc.vector.tensor_max(acc2[:], acc2[:], c[:])
                    ctiles = [c[:, 0:GW], c[:, GW:N]]
                    # dist1 funnel: fp16 fold chain (2x_1p) then one reduce.
                    if "nofold" in probe:
                        continue
                    jk = junkpool.tile([P, GW], FP16)
                    nc.vector.tensor_max(jk[:], ctiles[0], ctiles[1])
                    nc.vector.tensor_max(
                        jk[:, 0:1024], jk[:, 0:1024], jk[:, 1024:2048]
                    )
                    nc.vector.tensor_max(
                        w1[:, t * 512:(t + 1) * 512], jk[:, 0:512], jk[:, 512:1024]
                    )
                # one batched reduce for all m-tiles: [P, mt, 512] -> [P, mt]
                nc.vector.tensor_reduce(
                    d1[:, 0:mt],
                    w1[:, 0:mt * 512].rearrange("p (t q) -> p t q", t=mt),
                    axis=mybir.AxisListType.X,
                    op=mybir.AluOpType.max,
                )

        # dist2 partition-axis max is finished on host: ship acc2 as-is.
        nc.sync.dma_start(out_d[:], d1[:])
        nc.sync.dma_start(acc_d[:], acc2[:])

    nc.compile()
    return nc


def _np3split(v: np.ndarray):
    """3-way bf16 split of float64/float32 array v: returns (h1,h2,h3) bf16."""
    v = v.astype(np.float64)
    h1 = v.astype(ml_dtypes.bfloat16)
    r1 = v - h1.astype(np.float64)
    h2 = r1.astype(ml_dtypes.bfloat16)
    r2 = r1 - h2.astype(np.float64)
    h3 = r2.astype(ml_dtypes.bfloat16)
    return h1, h2, h3


def make_inputs(x: np.ndarray, y: np.ndarray):
    """Host prep: augmented, 3-way-split, K-stacked operands per core."""
    x = np.asarray(x, dtype=np.float32)
    y = np.asarray(y, dtype=np.float32)
    x64 = x.astype(np.float64)
    y64 = y.astype(np.float64)
    x2 = (x64 * x64).sum(-1)  # [B,N]
    y2 = (y64 * y64).sum(-1)  # [B,M]

    # xhat [B,DA,N], yhat [B,DA,M]
    xhat = np.empty((B, DA, N), np.float64)
    xhat[:, 0:3, :] = x64.transpose(0, 2, 1)
    xhat[:, 3, :] = 1.0
    xhat[:, 4, :] = x2
    yhat = np.empty((B, DA, M), np.float64)
    yhat[:, 0:3, :] = 2.0 * y64.transpose(0, 2, 1)
    yhat[:, 3, :] = -y2
    yhat[:, 4, :] = -1.0

    xh = _np3split(xhat)  # each [B,DA,N] bf16
    yh = _np3split(yhat)

    # K-stack: all 9 (i,j) products
    xs = np.empty((B, KS, N), ml_dtypes.bfloat16)
    ys = np.empty((B, KS, M), ml_dtypes.bfloat16)
    blk = 0
    for i in range(3):
        for j in range(3):
            ys[:, blk * DA:(blk + 1) * DA, :] = yh[i]
            xs[:, blk * DA:(blk + 1) * DA, :] = xh[j]
            blk += 1

    in_maps = []
    for c in range(8):
        b, h = c // 2, c % 2
        in_maps.append({
            "ys": np.ascontiguousarray(ys[b, :, h * 2048:(h + 1) * 2048]),
            "xs": np.ascontiguousarray(xs[b]),
        })
    return in_maps


def combine(results):
    """Host combine: per core "out" [128,16] fp32, "acc" [128,4096] fp16."""
    smax1 = np.empty((B, M), np.float64)  # max_n s  (dist1 dir)
    smax2 = np.full((B, N), -np.inf, np.float64)  # max_m s (dist2 dir)
    for c in range(8):
        b, h = c // 2, c % 2
        d1 = np.asarray(results[c]["out"], np.float64)  # [128,16]: col t, m=h*2048+t*128+p
        smax1[b, h * 2048:(h + 1) * 2048] = d1.T.reshape(-1)
        acc = np.asarray(results[c]["acc"]).astype(np.float64)  # [128, 4096]: col = n
        smax2[b] = np.maximum(smax2[b], acc.max(axis=0))
    d2min_m = np.maximum(-smax1, 0.0)
    d2min_n = np.maximum(-smax2, 0.0)
    loss = np.sqrt(d2min_m).mean() + np.sqrt(d2min_n).mean()
    return np.float32(loss)


_CACHE = {}


def kernel(x, y):
    if "nc" not in _CACHE:
        _CACHE["nc"] = build_program(repeat=1)
    nc = _CACHE["nc"]
    in_maps = make_inputs(x, y)
    res = run_bass_kernel_spmd(nc, in_maps, list(range(8)))
    return combine(res.results)
